# revision 9
# baseline (speedup 1.0000x reference)
"""Two-layer GAT (PyG GATConv semantics) on 8 Trainium2 NeuronCores — v2.

Strategy (vs the per-edge-projection v1): project each node ONCE, then
deliver per-edge operands with batched SWDGE dma_gather row-gathers.

Layer 1 (per core, nodes sharded by dst):
  - Project the core's ~44k unique source nodes: G1loc[u] = h1 row (256B).
  - Per 4-block group, two dma_gathers (table split at 32768 for int16
    indices) fetch h1[src] for every edge slot. Attention logits
    lrelu(asrc1[src]+adst1[dst]) are host-computed (0.1% of model FLOPs)
    and streamed; exp runs on ACT with a broadcast-expand so the DVE
    multiply p*h runs on dense operands.
  - Aggregation: per 128-slot chunk, one PE matmul psA += mask^T @
    [p*h | p] with host-streamed fp8 masks (mixed fp8 x bf16 matmul).
  - Epilogue: normalize, bias, ELU(+1 shift folded into downstream
    constants), project with W2e -> T2 rows [h2|asrc2|adst2].

Exchange: AllGather of the 18-wide T2 table (1.8MB), expanded locally
into a 256B-row table T2pad for gathering.

Layer 2 (single head):
  - Edge slots grouped into 4-slot single-dst windows; adst2[dst] is
    delivered by a small extra row-gather of each window's dst row
    (A2row), read at a fixed stride — no transposes, no scatter.
  - p2 = exp(lrelu(asrc2+adst2+bias)) per slot (pad slots get -30000
    bias and vanish); windows reduce on DVE (17-wide payload), then a
    few window-level mask matmuls accumulate per-dst sums.

All shapes are uniform across cores (global maxima) so the single SPMD
program is valid; per-core data (indices, masks, logits) comes via
DRAM parameters.
"""
import math

import numpy as np
import ml_dtypes

import concourse.bass as bass
import concourse.mybir as mybir
import concourse.tile as tile
from concourse import bacc, library_config
from concourse.bass_utils import run_bass_kernel_spmd

# problem shape (hardcoded per spec)
N = 50000
E = 800000
NFEAT = 256
F1 = 128            # HEADS * NHID
HEADS = 8
NHID = 16
NCLASS = 16
NEG = 0.2

NCORES = 8
NB = 49             # 128-dst blocks per core
PN = NB * 128       # 6272 nodes per core
VN = NCORES * PN    # 50176 virtual nodes
SPLIT = 32768       # int16 gather index limit
GS = 4              # blocks per gather group
WS = 4              # window size (layer-2 single-dst windows)
NEGINF = -30000.0

F32 = mybir.dt.float32
BF16 = mybir.dt.bfloat16
FP8 = mybir.dt.float8e4
I16 = mybir.dt.int16

RW1 = F1 + HEADS    # 136: [p*h1 | p]
RW2 = NCLASS + 1    # 17: [p2*h2 | p2]

_nc_cache = {}


def _wrap_idx(seq):
    """int16 gather-index layout: index i at partition i%16, col i//16,
    replicated across the 8 groups of 16 partitions."""
    seq = np.asarray(seq, np.int16)
    assert len(seq) % 16 == 0
    w = seq.reshape(-1, 16).T  # [16, n/16]
    return np.tile(w, (8, 1))


def _lrelu(v):
    return np.where(v > 0, v, NEG * v)


def _host_prep(x, edge_index, W1, att_src1, att_dst1, b1, W2, att_src2,
               att_dst2, b2):
    x = np.asarray(x, np.float32)
    W1 = np.asarray(W1, np.float32)
    att_src1 = np.asarray(att_src1, np.float32)
    att_dst1 = np.asarray(att_dst1, np.float32)
    b1 = np.asarray(b1, np.float32)
    W2 = np.asarray(W2, np.float32)
    att_src2 = np.asarray(att_src2, np.float32)
    att_dst2 = np.asarray(att_dst2, np.float32)
    b2 = np.asarray(b2, np.float32)
    ei = np.asarray(edge_index).astype(np.int64)

    src = np.concatenate([ei[0], np.arange(N, dtype=np.int64)])
    dst = np.concatenate([ei[1], np.arange(N, dtype=np.int64)])

    # layer-1 attention coefficients on host (tiny fraction of FLOPs)
    W1r = W1.reshape(NFEAT, HEADS, NHID)
    Ws = np.einsum("khc,hc->kh", W1r, att_src1)
    Wd = np.einsum("khc,hc->kh", W1r, att_dst1)
    As = x @ Ws
    Ad = x @ Wd
    LL = _lrelu(As[src] + Ad[dst]).astype(np.float32)  # [Etot, 8]

    W2e = np.concatenate(
        [W2, (W2 @ att_src2[0])[:, None], (W2 @ att_dst2[0])[:, None]], axis=1
    )  # [128, 18]
    bias_real = 0.0
    b2eff = b2

    x_bf = x.astype(ml_dtypes.bfloat16)

    groups = [(g * GS, min((g + 1) * GS, NB))
              for g in range(math.ceil(NB / GS))]
    NG = len(groups)

    core_of = dst // PN
    cores = []
    for k in range(NCORES):
        m = core_of == k
        es = src[m]
        edl = dst[m] - k * PN          # local dst 0..PN-1
        lg = LL[m]
        uniq, uinv = np.unique(es, return_inverse=True)
        blk = edl // 128
        dstl = edl % 128
        cores.append(dict(es=es, u=uinv, uniq=uniq, blk=blk, dstl=dstl,
                          lg=lg))

    U_pad = 128 * math.ceil(max(len(c["uniq"]) for c in cores) / 128)

    # ---- pass 1: per core, per block: L1 edge splits and L2 windows ----
    for c in cores:
        order = np.argsort(c["blk"] * 128 + c["dstl"], kind="stable")
        for f in ("es", "u", "blk", "dstl", "lg"):
            c[f] = c[f][order]
        # per block boundaries
        c["bbounds"] = np.searchsorted(c["blk"], np.arange(NB + 1))
        # L1: per block lo/hi edge indices (u < SPLIT)
        l1lo, l1hi = [], []
        # L2: per block window list: (dstl, [edge idx...]) per region
        w2lo, w2hi = [], []
        for b in range(NB):
            e0, e1 = c["bbounds"][b], c["bbounds"][b + 1]
            eb = np.arange(e0, e1)
            lo = eb[c["u"][e0:e1] < SPLIT]
            hi = eb[c["u"][e0:e1] >= SPLIT]
            l1lo.append(lo)
            l1hi.append(hi)
            # windows: per dst within block
            dl = c["dstl"][e0:e1]
            vsrc = c["es"][e0:e1]
            wlo, whi = [], []
            db = np.searchsorted(dl, np.arange(129))
            for d in range(128):
                de = eb[db[d]:db[d + 1]]
                if len(de) == 0:
                    continue
                dv = vsrc[db[d]:db[d + 1]]
                delo = de[dv < SPLIT]
                dehi = de[dv >= SPLIT]
                for i in range(0, len(delo), WS):
                    wlo.append((d, delo[i:i + WS]))
                for i in range(0, len(dehi), WS):
                    whi.append((d, dehi[i:i + WS]))
            w2lo.append(wlo)
            w2hi.append(whi)
        c["l1lo"], c["l1hi"] = l1lo, l1hi
        c["w2lo"], c["w2hi"] = w2lo, w2hi

    # ---- global uniform shapes ----
    LO1 = np.zeros(NG, np.int64)
    HI1 = np.zeros(NG, np.int64)
    K2L = np.zeros(NG, np.int64)
    K2H = np.zeros(NG, np.int64)
    # per (g, local b): chunk index ranges (global unions)
    rng1lo = [[None] * (be - bs) for (bs, be) in groups]
    rng1hi = [[None] * (be - bs) for (bs, be) in groups]
    rng2lo = [[None] * (be - bs) for (bs, be) in groups]
    rng2hi = [[None] * (be - bs) for (bs, be) in groups]

    for g, (bs, be) in enumerate(groups):
        nb = be - bs
        for lst_name, TOT, RNG in (("l1lo", LO1, rng1lo),
                                   ("l1hi", HI1, rng1hi),
                                   ("w2lo", K2L, rng2lo),
                                   ("w2hi", K2H, rng2hi)):
            cmaxcols = 0
            st = [[None, None] for _ in range(nb)]
            for c in cores:
                cum = 0
                for bl in range(nb):
                    n = len(c[lst_name][bs + bl])
                    if n > 0:
                        c0, c1 = cum // 128, math.ceil((cum + n) / 128)
                        if st[bl][0] is None:
                            st[bl] = [c0, c1]
                        else:
                            st[bl][0] = min(st[bl][0], c0)
                            st[bl][1] = max(st[bl][1], c1)
                    cum += n
                cmaxcols = max(cmaxcols, math.ceil(cum / 128))
            TOT[g] = max([cmaxcols]
                         + [s[1] for s in st if s[1] is not None] + [1])
            RNG[g] = [(int(s[0]), int(s[1])) if s[0] is not None else (0, 0)
                      for s in st]

    # mask-matmul pair lists (uniform across cores)
    pairs1 = []  # per g: list of (local_b, chunk_col_in_group, is_first, is_last)
    pairs2 = []
    for g, (bs, be) in enumerate(groups):
        nb = be - bs
        p1, p2_ = [], []
        for bl in range(nb):
            ch = (list(range(*rng1lo[g][bl]))
                  + [int(LO1[g]) + cc for cc in range(*rng1hi[g][bl])])
            p1.append([int(v) for v in ch])
            ch2 = (list(range(*rng2lo[g][bl]))
                   + [int(K2L[g]) + kk for kk in range(*rng2hi[g][bl])])
            p2_.append([int(v) for v in ch2])
        pairs1.append(p1)
        pairs2.append(p2_)

    meta = dict(groups=groups, U_pad=int(U_pad),
                LO1=[int(v) for v in LO1], HI1=[int(v) for v in HI1],
                K2L=[int(v) for v in K2L], K2H=[int(v) for v in K2H],
                pairs1=pairs1, pairs2=pairs2)

    # ---- pass 2: emit per-core arrays ----
    in_maps = []
    W1bf = W1.astype(ml_dtypes.bfloat16)
    W2ebf = W2e.astype(ml_dtypes.bfloat16)
    b1r = np.tile(b1[None, :], (128, 1)).astype(np.float32)
    b2r = np.tile(b2eff[None, :], (128, 1)).astype(np.float32)

    for k, c in enumerate(cores):
        xT = np.zeros((NFEAT, U_pad), ml_dtypes.bfloat16)
        nu = len(c["uniq"])
        xT[:, :nu] = x_bf[c["uniq"]].T

        idx1_parts, lg1_parts, msk1_parts = [], [], []
        idx2_parts, bias2_parts, mskw_parts, idxa2_parts = [], [], [], []

        for g, (bs, be) in enumerate(groups):
            nb = be - bs
            # ----- L1 slots -----
            ncols = LO1[g] + HI1[g]
            uoff = np.zeros(ncols * 128, np.int64)
            lg8 = np.full((ncols * 128, HEADS), NEGINF, np.float32)
            blkid = np.full(ncols * 128, -1, np.int64)
            dstl = np.zeros(ncols * 128, np.int64)
            pos = 0
            for reg, lst_name, base in (("lo", "l1lo", 0),
                                        ("hi", "l1hi", LO1[g] * 128)):
                pos = base
                for bl in range(nb):
                    ee = c[lst_name][bs + bl]
                    s = slice(pos, pos + len(ee))
                    uoff[s] = c["u"][ee] - (SPLIT if reg == "hi" else 0)
                    lg8[s] = c["lg"][ee]
                    blkid[s] = bl
                    dstl[s] = c["dstl"][ee]
                    pos += len(ee)
            idx1_parts.append(_wrap_idx(uoff[:LO1[g] * 128]))
            idx1_parts.append(_wrap_idx(uoff[LO1[g] * 128:]))
            # Lg1 layout [128, ncols*8]: slot (p, col) = seq col*128+p
            lgt = lg8.reshape(ncols, 128, HEADS).transpose(1, 0, 2)
            lg1_parts.append(
                lgt.reshape(128, ncols * HEADS).astype(ml_dtypes.bfloat16))
            # masks per pair
            blk2 = blkid.reshape(ncols, 128)
            dst2 = dstl.reshape(ncols, 128)
            for bl in range(nb):
                for cc in pairs1[g][bl]:
                    m = np.zeros((128, 128), np.float32)
                    sel = blk2[cc] == bl
                    m[np.nonzero(sel)[0], dst2[cc][sel]] = 1.0
                    msk1_parts.append(m.astype(ml_dtypes.float8_e4m3))

            # ----- L2 windows -----
            K2 = K2L[g] + K2H[g]
            nwslots = K2 * 128
            voff = np.zeros((nwslots, WS), np.int64)
            bias = np.full((nwslots, WS), NEGINF, np.float32)
            wblk = np.full(nwslots, -1, np.int64)
            wd = np.zeros(nwslots, np.int64)
            for reg, lst_name, base in (("lo", "w2lo", 0),
                                        ("hi", "w2hi", K2L[g] * 128)):
                pos = base
                for bl in range(nb):
                    for (d, ee) in c[lst_name][bs + bl]:
                        voff[pos, :len(ee)] = c["es"][ee] - (
                            SPLIT if reg == "hi" else 0)
                        bias[pos, :len(ee)] = bias_real
                        wblk[pos] = bl
                        wd[pos] = d
                        pos += 1
            # slot order for gather: i = col*128 + p, col = 4k + cw,
            # window w = k*128+p -> voff[w, cw]
            v3 = voff.reshape(K2, 128, WS)       # [k, p, cw]
            arr = v3.transpose(0, 2, 1).reshape(K2 * WS * 128)  # [(k,cw),p]
            idx2_parts.append(_wrap_idx(arr[:K2L[g] * WS * 128]))
            idx2_parts.append(_wrap_idx(arr[K2L[g] * WS * 128:]))
            bias3 = bias.reshape(K2, 128, WS).transpose(0, 2, 1)
            bias2_parts.append(
                bias3.reshape(K2 * WS, 128).T.astype(ml_dtypes.bfloat16))
            # A2 gather: window dst rows (core-local ids), order i = k*128+p
            a2 = (wblk.clip(0) + bs) * 128 + wd
            a2[wblk < 0] = 0
            idxa2_parts.append(_wrap_idx(a2))
            # window masks
            wblk2 = wblk.reshape(K2, 128)
            wd2 = wd.reshape(K2, 128)
            for bl in range(nb):
                for kk in pairs2[g][bl]:
                    m = np.zeros((128, 128), np.float32)
                    sel = wblk2[kk] == bl
                    m[np.nonzero(sel)[0], wd2[kk][sel]] = 1.0
                    mskw_parts.append(m.astype(ml_dtypes.float8_e4m3))

        m = dict(
            xTsrc=np.ascontiguousarray(xT),
            idx1=np.ascontiguousarray(np.hstack(idx1_parts)),
            Lg1=np.ascontiguousarray(np.hstack(lg1_parts)),
            msk1=np.ascontiguousarray(
                np.concatenate(msk1_parts, 1) if msk1_parts else
                np.zeros((128, 128), ml_dtypes.float8_e4m3)),
            idx2=np.ascontiguousarray(np.hstack(idx2_parts)),
            bias2=np.ascontiguousarray(np.hstack(bias2_parts)),
            idxA2=np.ascontiguousarray(np.hstack(idxa2_parts)),
            mskW=np.ascontiguousarray(np.concatenate(mskw_parts, 1)),
            W1a=np.ascontiguousarray(W1bf[0:128]),
            W1b=np.ascontiguousarray(W1bf[128:256]),
            W2e=W2ebf,
            b1r=b1r, b2r=b2r,
        )
        in_maps.append(m)
    return in_maps, meta


def _build(meta, in_shapes):
    groups = meta["groups"]
    U_pad = meta["U_pad"]
    LO1, HI1 = meta["LO1"], meta["HI1"]
    K2L, K2H = meta["K2L"], meta["K2H"]
    pairs1, pairs2 = meta["pairs1"], meta["pairs2"]
    NG = len(groups)

    nc = bacc.Bacc("TRN2", target_bir_lowering=False, debug=False,
                   num_devices=NCORES)

    dp = {}
    for name, shp, dt in in_shapes:
        dp[name] = nc.declare_dram_parameter(name, list(shp), dt,
                                             isOutput=False)
    out_d = nc.declare_dram_parameter("out", [PN, NCLASS], F32, isOutput=True)

    G1loc = nc.dram_tensor("G1loc", [U_pad, F1], BF16)
    G2s = nc.dram_tensor("G2s", [PN, NCLASS + 2], BF16)
    G2f = nc.dram_tensor("G2f", [VN, NCLASS + 2], BF16, addr_space="Shared")
    T2pad = nc.dram_tensor("T2pad", [VN, F1], BF16)
    T2own = nc.dram_tensor("T2own", [PN, F1], BF16)

    AF = mybir.ActivationFunctionType
    OP = mybir.AluOpType
    NU = U_pad // 128

    with tile.TileContext(nc) as tc:
        with tc.tile_pool(name="consts", bufs=1) as cw:
            nc.gpsimd.load_library(library_config.mlp)
            w1a = cw.tile([128, F1], BF16)
            nc.sync.dma_start(out=w1a[:, :], in_=dp["W1a"][:, :])
            w1b = cw.tile([128, F1], BF16)
            nc.sync.dma_start(out=w1b[:, :], in_=dp["W1b"][:, :])
            w2e = cw.tile([F1, NCLASS + 2], BF16)
            nc.sync.dma_start(out=w2e[:, :], in_=dp["W2e"][:, :])
            b1r = cw.tile([128, F1], F32)
            nc.sync.dma_start(out=b1r[:, :], in_=dp["b1r"][:, :])
            b2r = cw.tile([128, NCLASS], F32)
            nc.sync.dma_start(out=b2r[:, :], in_=dp["b2r"][:, :])
            identb = cw.tile([128, 128], BF16)
            from concourse.masks import make_identity
            make_identity(nc, identb[:, :])

            # ---- projection: G1loc[u] = (x_u @ W1) ----
            with (
                tc.tile_pool(name="projx", bufs=3) as px,
                tc.tile_pool(name="projo", bufs=3) as po,
                tc.tile_pool(name="projps", bufs=2, space="PSUM") as pps,
            ):
                for gi in range(NU):
                    cs = slice(gi * 128, (gi + 1) * 128)
                    xc0 = px.tile([128, 128], BF16, tag="xc0")
                    nc.sync.dma_start(out=xc0[:, :], in_=dp["xTsrc"][0:128, cs])
                    xc1 = px.tile([128, 128], BF16, tag="xc1")
                    nc.sync.dma_start(out=xc1[:, :],
                                      in_=dp["xTsrc"][128:256, cs])
                    ps = pps.tile([128, F1], F32, tag="ps")
                    nc.tensor.matmul(ps[:, :], lhsT=xc0[:, :], rhs=w1a[:, :],
                                     start=True, stop=False)
                    nc.tensor.matmul(ps[:, :], lhsT=xc1[:, :], rhs=w1b[:, :],
                                     start=False, stop=True)
                    hb = po.tile([128, F1], BF16, tag="hb")
                    if gi % 2 == 0:
                        nc.scalar.copy(out=hb[:, :], in_=ps[:, :])
                    else:
                        nc.vector.tensor_copy(out=hb[:, :], in_=ps[:, :])
                    nc.sync.dma_start(out=G1loc[cs, :], in_=hb[:, :])

            # ---- S2: layer 1 ----
            i1off = 0   # idx1 column offset
            lgoff = 0
            m1off = 0   # msk1 column offset (units of 128)
            with (
                tc.tile_pool(name="g1p", bufs=2) as g1p,
                tc.tile_pool(name="s2st", bufs=2) as s2st,
                tc.tile_pool(name="s2w", bufs=2) as s2w,
                tc.tile_pool(name="s2e", bufs=2) as s2e,
                tc.tile_pool(name="psA", bufs=2, space="PSUM") as psAp,
                tc.tile_pool(name="scr2", bufs=2, space="PSUM") as scr2,
            ):
                for g, (bs, be) in enumerate(groups):
                    nb = be - bs
                    CL, CH = LO1[g], HI1[g]
                    CC = CL + CH
                    npair = sum(len(p) for p in pairs1[g])

                    it = s2st.tile([128, CC * 8], I16, tag="idx")
                    nc.sync.dma_start(out=it[:, :],
                                      in_=dp["idx1"][:, i1off:i1off + CC * 8])
                    lgt = s2st.tile([128, CC * HEADS], BF16, tag="lg")
                    nc.sync.dma_start(
                        out=lgt[:, :],
                        in_=dp["Lg1"][:, lgoff:lgoff + CC * HEADS])
                    mt = s2st.tile([128, npair * 128], FP8, tag="msk")
                    nc.sync.dma_start(
                        out=mt[:, :],
                        in_=dp["msk1"][:, m1off:m1off + npair * 128])

                    gt = g1p.tile([128, CC * F1], BF16, tag="g1")
                    g3 = gt[:, :].rearrange("p (c e) -> p c e", e=F1)
                    nc.gpsimd.dma_gather(
                        g3[:, 0:CL, :], G1loc[:, :], it[:, 0:CL * 8],
                        CL * 128, CL * 128, F1, single_packet=False)
                    nc.gpsimd.dma_gather(
                        g3[:, CL:CC, :], G1loc[SPLIT:, :], it[:, CL * 8:],
                        CH * 128, CH * 128, F1, single_packet=False)

                    # p expanded: pexp[p, c, h, 16] = exp(Lg1[p, c, h])
                    pexp = s2w.tile([128, CC * F1], BF16, tag="pexp")
                    nc.scalar.activation(
                        out=pexp[:, :].rearrange("p (c h x) -> p c h x",
                                                 h=HEADS, x=NHID),
                        in_=lgt[:, :].rearrange("p (c h) -> p c h", h=HEADS)
                        .unsqueeze(3).to_broadcast([128, CC, HEADS, NHID]),
                        func=AF.Exp)
                    rhs = s2w.tile([128, CC * RW1], BF16, tag="rhs")
                    r3 = rhs[:, :].rearrange("p (c j) -> p c j", j=RW1)
                    nc.vector.tensor_tensor(
                        out=r3[:, :, 0:F1],
                        in0=g3[:, :, :], in1=pexp[:, :].rearrange(
                            "p (c e) -> p c e", e=F1),
                        op=OP.mult)
                    nc.scalar.copy(
                        out=r3[:, :, F1:RW1].unsqueeze(3),
                        in_=pexp[:, :].rearrange("p (c h x) -> p c h x",
                                                 h=HEADS, x=NHID)[:, :, :, 0:1])

                    pi = 0
                    for bl in range(nb):
                        ch = pairs1[g][bl]
                        psA = psAp.tile([128, RW1], F32, tag="psA")
                        for i, cc in enumerate(ch):
                            nc.tensor.matmul(
                                psA[:, :],
                                lhsT=mt[:, (pi + i) * 128:(pi + i + 1) * 128],
                                rhs=rhs[:, cc * RW1:(cc + 1) * RW1],
                                start=(i == 0), stop=(i == len(ch) - 1))
                        pi += len(ch)

                        # epilogue for block bs+bl
                        b = bs + bl
                        sA = s2e.tile([128, RW1], F32, tag="sA")
                        nc.scalar.copy(out=sA[:, :], in_=psA[:, :])
                        den = s2e.tile([128, HEADS], F32, tag="den")
                        nc.vector.tensor_scalar_max(den[:, :],
                                                    sA[:, F1:RW1], 1e-30)
                        rec = s2e.tile([128, HEADS], F32, tag="rec")
                        nc.vector.reciprocal(out=rec[:, :], in_=den[:, :])
                        y = s2e.tile([128, F1], F32, tag="y")
                        nc.vector.tensor_tensor(
                            out=y[:, :].rearrange("p (h x) -> p h x", x=NHID),
                            in0=sA[:, 0:F1].rearrange("p (h x) -> p h x",
                                                      x=NHID),
                            in1=rec[:, :].unsqueeze(2).to_broadcast(
                                [128, HEADS, NHID]),
                            op=OP.mult)
                        nc.vector.tensor_tensor(out=y[:, :], in0=y[:, :],
                                                in1=b1r[:, :], op=OP.add)
                        ng = s2e.tile([128, F1], F32, tag="ng")
                        nc.vector.tensor_scalar_min(ng[:, :], y[:, :], 0.0)
                        en = s2e.tile([128, F1], F32, tag="en")
                        nc.scalar.activation(out=en[:, :], in_=ng[:, :],
                                             func=AF.Exp)
                        h1s = s2e.tile([128, F1], BF16, tag="h1s")
                        ps_ = s2e.tile([128, F1], F32, tag="pos")
                        nc.vector.tensor_scalar_max(ps_[:, :], y[:, :], 0.0)
                        nc.vector.tensor_tensor(out=ps_[:, :], in0=ps_[:, :],
                                                in1=en[:, :], op=OP.add)
                        nc.vector.tensor_scalar_add(h1s[:, :], ps_[:, :],
                                                    -1.0)
                        psT = scr2.tile([128, 128], BF16, tag="psT")
                        nc.tensor.transpose(out=psT[:, :], in_=h1s[:, :],
                                            identity=identb[:, :])
                        h1t = s2e.tile([128, 128], BF16, tag="h1t")
                        nc.scalar.copy(out=h1t[:, :], in_=psT[:, :])
                        ps2 = scr2.tile([128, NCLASS + 2], F32, tag="ps2")
                        nc.tensor.matmul(ps2[:, :], lhsT=h1t[:, :],
                                         rhs=w2e[:, :], start=True, stop=True)
                        t2r = s2e.tile([128, NCLASS + 2], BF16, tag="t2r")
                        nc.scalar.copy(out=t2r[:, :], in_=ps2[:, :])
                        rs = slice(b * 128, (b + 1) * 128)
                        nc.sync.dma_start(out=G2s[rs, :], in_=t2r[:, :])
                        nc.sync.dma_start(out=T2own[rs, 0:NCLASS + 2],
                                          in_=t2r[:, :])
                    i1off += CC * 8
                    lgoff += CC * HEADS
                    m1off += npair * 128

            # ---- exchange + expand ----
            nc.gpsimd.collective_compute(
                "AllGather", mybir.AluOpType.bypass,
                ins=[G2s[:, :]], outs=[G2f[:, :]],
                replica_groups=[list(range(NCORES))])
            with tc.tile_pool(name="expand", bufs=1) as ep:
                NR = VN // 128
                stg = ep.tile([128, NR * (NCLASS + 2)], BF16)
                nc.sync.dma_start(
                    out=stg[:, :].rearrange("p (r j) -> p r j",
                                            j=NCLASS + 2),
                    in_=G2f[:, :].rearrange("(r p) j -> p r j", p=128))
                nc.sync.dma_start(
                    out=T2pad[:, :].rearrange("(r p) e -> p r e",
                                              p=128)[:, :, 0:NCLASS + 2],
                    in_=stg[:, :].rearrange("p (r j) -> p r j",
                                            j=NCLASS + 2))

            # ---- S3: layer 2 ----
            i2off = 0
            boff = 0
            a2off = 0
            mwoff = 0
            with (
                tc.tile_pool(name="g2p", bufs=2) as g2p,
                tc.tile_pool(name="s3st", bufs=2) as s3st,
                tc.tile_pool(name="s3w", bufs=2) as s3w,
                tc.tile_pool(name="s3e", bufs=2) as s3e,
                tc.tile_pool(name="psB", bufs=2, space="PSUM") as psBp,
            ):
                for g, (bs, be) in enumerate(groups):
                    nb = be - bs
                    KL, KH = K2L[g], K2H[g]
                    K2 = KL + KH
                    C2 = K2 * WS
                    npair = sum(len(p) for p in pairs2[g])

                    it = s3st.tile([128, C2 * 8], I16, tag="idx")
                    nc.sync.dma_start(out=it[:, :],
                                      in_=dp["idx2"][:, i2off:i2off + C2 * 8])
                    ia = s3st.tile([128, K2 * 8], I16, tag="idxa")
                    nc.sync.dma_start(out=ia[:, :],
                                      in_=dp["idxA2"][:, a2off:a2off + K2 * 8])
                    bt = s3st.tile([128, C2], BF16, tag="bias")
                    nc.sync.dma_start(out=bt[:, :],
                                      in_=dp["bias2"][:, boff:boff + C2])
                    mw = s3st.tile([128, npair * 128], FP8, tag="mskw")
                    nc.sync.dma_start(
                        out=mw[:, :],
                        in_=dp["mskW"][:, mwoff:mwoff + npair * 128])

                    g2t = g2p.tile([128, C2 * F1], BF16, tag="g2")
                    g23 = g2t[:, :].rearrange("p (c e) -> p c e", e=F1)
                    nc.gpsimd.dma_gather(
                        g23[:, 0:KL * WS, :], T2pad[:, :],
                        it[:, 0:KL * WS * 8],
                        KL * WS * 128, KL * WS * 128, F1, single_packet=False)
                    nc.gpsimd.dma_gather(
                        g23[:, KL * WS:C2, :], T2pad[SPLIT:, :],
                        it[:, KL * WS * 8:],
                        KH * WS * 128, KH * WS * 128, F1, single_packet=False)
                    a2t = g2p.tile([128, K2 * F1], BF16, tag="a2")
                    nc.gpsimd.dma_gather(
                        a2t[:, :].rearrange("p (c e) -> p c e", e=F1),
                        T2own[:, :], ia[:, :],
                        K2 * 128, K2 * 128, F1, single_packet=False)

                    # logits: asrc2[src] + adst2[dst] + bias
                    lg2 = s3w.tile([128, C2], F32, tag="lg2")
                    nc.vector.tensor_tensor(
                        out=lg2[:, :].rearrange("p (k c) -> p k c", c=WS),
                        in0=g23[:, :, NCLASS:NCLASS + 1]
                        .rearrange("p (k c) x -> p k (c x)", c=WS),
                        in1=a2t[:, :].rearrange("p (k e) -> p k e",
                                                e=F1)[:, :, NCLASS + 1:NCLASS + 2]
                        .to_broadcast([128, K2, WS]),
                        op=OP.add)
                    nc.vector.tensor_tensor(out=lg2[:, :], in0=lg2[:, :],
                                            in1=bt[:, :], op=OP.add)
                    lr2 = s3w.tile([128, C2], F32, tag="lr2")
                    nc.scalar.activation(out=lr2[:, :], in_=lg2[:, :],
                                         func=AF.Lrelu, alpha=NEG)
                    p2 = s3w.tile([128, C2], BF16, tag="p2")
                    nc.scalar.activation(out=p2[:, :], in_=lr2[:, :],
                                         func=AF.Exp)

                    # V4[p, k, j, c]: j in 0..16 -> p2*h2 | p2
                    V4 = s3w.tile([128, K2 * RW2 * WS], F32, tag="v4")
                    v4r = V4[:, :].rearrange("p (k j c) -> p k j c",
                                             j=RW2, c=WS)
                    nc.vector.tensor_tensor(
                        out=v4r[:, :, 0:NCLASS, :],
                        in0=g2t[:, :].rearrange("p (k c e) -> p k e c",
                                                c=WS, e=F1)[:, :, 0:NCLASS, :],
                        in1=p2[:, :].rearrange("p (k c) -> p k c", c=WS)
                        .unsqueeze(2).to_broadcast([128, K2, NCLASS, WS]),
                        op=OP.mult)
                    nc.vector.tensor_copy(
                        out=v4r[:, :, NCLASS:RW2, :],
                        in_=p2[:, :].rearrange("p (k c) -> p k c", c=WS)
                        .unsqueeze(2))
                    R = s3w.tile([128, K2 * RW2], F32, tag="R")
                    nc.vector.tensor_reduce(
                        out=R[:, :].unsqueeze(2),
                        in_=V4[:, :].rearrange("p (s c) -> p s c", c=WS),
                        axis=mybir.AxisListType.X, op=OP.add)
                    Rb = s3w.tile([128, K2 * RW2], BF16, tag="Rb")
                    nc.scalar.copy(out=Rb[:, :], in_=R[:, :])

                    pi = 0
                    for bl in range(nb):
                        ch = pairs2[g][bl]
                        psB = psBp.tile([128, RW2], F32, tag="psB")
                        for i, kk in enumerate(ch):
                            nc.tensor.matmul(
                                psB[:, :],
                                lhsT=mw[:, (pi + i) * 128:(pi + i + 1) * 128],
                                rhs=Rb[:, kk * RW2:(kk + 1) * RW2],
                                start=(i == 0), stop=(i == len(ch) - 1))
                        pi += len(ch)

                        b = bs + bl
                        sB = s3e.tile([128, RW2], F32, tag="sB")
                        nc.scalar.copy(out=sB[:, :], in_=psB[:, :])
                        dn = s3e.tile([128, 1], F32, tag="dn")
                        nc.vector.tensor_scalar_max(
                            dn[:, :], sB[:, NCLASS:RW2], 1e-30)
                        rc = s3e.tile([128, 1], F32, tag="rc")
                        nc.vector.reciprocal(out=rc[:, :], in_=dn[:, :])
                        o = s3e.tile([128, NCLASS], F32, tag="o")
                        nc.vector.tensor_tensor(
                            out=o[:, :], in0=sB[:, 0:NCLASS],
                            in1=rc[:, :].to_broadcast([128, NCLASS]),
                            op=OP.mult)
                        nc.vector.tensor_tensor(out=o[:, :], in0=o[:, :],
                                                in1=b2r[:, :], op=OP.add)
                        nc.sync.dma_start(
                            out=out_d[b * 128:(b + 1) * 128, :], in_=o[:, :])
                    i2off += C2 * 8
                    boff += C2
                    a2off += K2 * 8
                    mwoff += npair * 128

    nc.compile()
    return nc


def _meta_key(meta, in_shapes):
    import json
    return json.dumps([meta["groups"], meta["U_pad"], meta["LO1"],
                       meta["HI1"], meta["K2L"], meta["K2H"], meta["pairs1"],
                       meta["pairs2"], [list(map(str, s)) for s in in_shapes]])


_DT_MAP = {
    np.dtype(ml_dtypes.bfloat16): BF16,
    np.dtype(ml_dtypes.float8_e4m3): FP8,
    np.dtype(np.float32): F32,
    np.dtype(np.int16): I16,
}


def kernel(**inputs):
    in_maps, meta = _host_prep(**inputs)
    in_shapes = [(k, v.shape, _DT_MAP[v.dtype])
                 for k, v in sorted(in_maps[0].items())]
    key = _meta_key(meta, in_shapes)
    if key not in _nc_cache:
        _nc_cache[key] = _build(meta, in_shapes)
    nc = _nc_cache[key]
    res = run_bass_kernel_spmd(nc, in_maps, list(range(NCORES)))
    out = np.concatenate([res.results[k]["out"] for k in range(NCORES)],
                         axis=0)
    return np.ascontiguousarray(out[:N]).astype(np.float32)
